# revision 9
# baseline (speedup 1.0000x reference)
"""Trainium2 Bass kernel for ContextQueryAttention (BiDAF-style trilinear
attention). Data-parallel over batch across 8 NeuronCores (4 batches/core).

Per batch (c=1024 context rows, q=128 query rows, h=256 hidden):
  S[c,q]   = ctx@cw + (qry@qw)^T + (ctx*cqw)@qry^T + bias
  S_bar    = softmax_c(S); S_bar_bar = softmax_q(S)
  A        = S @ qry
  B        = S_bar @ (S_bar_bar^T @ ctx)
  out      = concat([ctx, A, ctx*A, ctx*B], -1)

v11: three-ring load/store split, merged exp, dense PE warmup.
  - qt_cq (= qry^T*cqw + cw) and s1 (= qry@qw + bias) precomputed on the
    host during input packing: no on-device preamble compute at all.
  - s1 applied as per-partition activation bias on the exp AND as a
    scalar add on the raw-S evac (kills the rank-1 rider matmuls).
  - zc (softmax_q denominators) via 8 tiny ones-column matmuls on the PE
    (kills the DVE reduce); e_sb = tr * rc via broadcast tensor_tensor
    (stride-0 in1), one op per 4-tile transpose wave.
  - ctx*B stored as fp8 (values are convex-combos of ctx, |x| << 240).
  - A, ctxA, ctxB in three separate contiguous HBM tensors; A stored
    right after its ACT evac, ctxA after Pool, ctxB after the deferred
    B phase (pairwise for the last batch to shorten the tail).
  - ALL inputs load up-front as back-to-back 1MB DMAs on the sync ring
    (batch 0 split in two contiguous tensors so compute starts sooner);
    every store queues behind them on the same ring, so the ring never
    mixes read/write directions at the HBM.
  - PE warmup: 7 dense N=512 matmuls on memset garbage (no DMA dep) so
    HAM un-throttles (1.2 -> 2.4 GHz) right as the first real matmul
    issues.
  - T/B of the last batch folded into the last iteration so only the
    final ctx*B products + stores trail the loop.
"""

import numpy as np

B, C, Q, H = 32, 1024, 128, 256
N_CORES = 8
BPC = B // N_CORES  # batches per core
P = 128
HC = H // P  # h chunks of 128
CT = C // P  # c tiles of 128
CCH = 512  # S^T free-dim chunk (1 PSUM bank of fp32)
NCC = C // CCH

_NC_CACHE = {}


def _build_kernel():
    import concourse.bacc as bacc
    import concourse.tile as tile
    from concourse import mybir
    from concourse.bass import broadcast_tensor_aps

    f32 = mybir.dt.float32
    bf16 = mybir.dt.bfloat16
    fp8 = mybir.dt.float8e4
    AF = mybir.ActivationFunctionType
    ALU = mybir.AluOpType

    nc = bacc.Bacc(trn_type="TRN2", target_bir_lowering=False, debug=False)
    # consts: qtw = [qt_cq cols (host-folded) | identity | ones-col] bf16
    QTW_W = HC * BPC * Q + P + 1
    qtw_d = nc.dram_tensor("qtw", [P, QTW_W], bf16, kind="ExternalInput").ap()
    qa_d = nc.dram_tensor("qa", [P, BPC * H], bf16, kind="ExternalInput").ap()
    # wv: per-batch s1 columns (qry@qw + bias), f32
    wv_d = nc.dram_tensor("wv", [P, BPC], f32, kind="ExternalInput").ap()
    # per-batch input: [ctxT cols | ctx cols] merged, 8KB/partition
    cin_d = nc.dram_tensor(
        "cin", [BPC, P, HC * C + CT * H], bf16, kind="ExternalInput"
    ).ap()
    # batch-0 input again as two separate contiguous tensors (fast startup)
    c0T_d = nc.dram_tensor("c0T", [P, HC * C], bf16, kind="ExternalInput").ap()
    c0N_d = nc.dram_tensor("c0N", [P, CT * H], bf16, kind="ExternalInput").ap()
    # outputs: A|ctxA merged per batch (one 1MB store), ctxB separate fp8
    oAC_d = nc.dram_tensor(
        "oAC", [BPC, P, 2 * CT * H], bf16, kind="ExternalOutput"
    ).ap()
    oB_d = nc.dram_tensor("oB", [BPC, P, CT * H], fp8, kind="ExternalOutput").ap()

    from contextlib import ExitStack

    with tile.TileContext(nc) as tc, ExitStack() as es:
        consts = es.enter_context(tc.tile_pool(name="consts", bufs=1))
        p_cin = es.enter_context(tc.tile_pool(name="p_cin", bufs=4))
        p_et = es.enter_context(tc.tile_pool(name="p_et", bufs=2))
        p_sr = es.enter_context(tc.tile_pool(name="p_sr", bufs=2))
        p_esb = es.enter_context(tc.tile_pool(name="p_esb", bufs=2))
        p_oAC = es.enter_context(tc.tile_pool(name="p_oAC", bufs=2))
        p_oB = es.enter_context(tc.tile_pool(name="p_oB", bufs=2))
        p_vec = es.enter_context(tc.tile_pool(name="p_vec", bufs=3))
        # PSUM banks: stp 2x[P,512]f32 (2) + tr 2x[P,4,128]bf16 (2)
        #             + tz 1x[P,264]f32 (1) + ab 3x[P,2,256]f32 (3)
        pp_st = es.enter_context(tc.tile_pool(name="pp_st", bufs=1, space="PSUM"))
        pp_tr = es.enter_context(tc.tile_pool(name="pp_tr", bufs=2, space="PSUM"))
        pp_tz = es.enter_context(tc.tile_pool(name="pp_tz", bufs=1, space="PSUM"))
        pp_ab = es.enter_context(tc.tile_pool(name="pp_ab", bufs=3, space="PSUM"))

        # ---- PE warmup spin on memset garbage (no DMA dependency): dense
        # transposes + N=512 matmuls (~4us) so HAM un-throttles early ----
        warm_src = consts.tile([P, P], bf16)
        nc.gpsimd.memset(warm_src, 1.0)
        warm_rhs = consts.tile([P, CCH], bf16)
        nc.vector.memset(warm_rhs, 1.0)
        for w in range(2):
            tr_warm = pp_tr.tile([P, 4, P], bf16, tag="tr8", name=f"warm{w}")
            for i in range(8):
                nc.tensor.matmul(
                    tr_warm[:, i % 4, :], lhsT=warm_src, rhs=warm_src,
                    is_transpose=True,
                )
        stp_w = pp_st.tile([P, NCC, CCH], f32, tag="stp", name="warm")
        for i in range(6):
            nc.tensor.matmul(
                stp_w[:, i % 2, :], lhsT=warm_src, rhs=warm_rhs, start=True, stop=True
            )

        # ---- const DMAs (qtw + s1 first; qa slots between batch-0 halves) ----
        qtw = consts.tile([P, QTW_W], bf16)
        nc.sync.dma_start(out=qtw, in_=qtw_d)
        wv = consts.tile([P, BPC], f32)
        nc.sync.dma_start(out=wv, in_=wv_d)
        qa_sb = consts.tile([P, BPC * H], bf16)
        qt_cq = qtw[:, 0 : HC * BPC * Q].rearrange("p (j bq) -> p j bq", j=HC)
        qa_all = qa_sb.rearrange("p (b h) -> p b h", b=BPC)
        idones = qtw[:, HC * BPC * Q :]  # [P, 129] = [I | 1]
        ident = idones[:, 0:P]
        ones_col = idones[:, P : P + 1]

        # ---- all input loads up-front, back-to-back on the sync ring:
        # the ring stays unidirectional (loads first, stores behind) ----
        def load_batch(b):
            cin_t = p_cin.tile([P, HC * C + CT * H], bf16, tag="cin", name=f"cin{b}")
            if b == 0:
                nc.scalar.dma_start(out=cin_t[:, 0 : HC * C], in_=c0T_d)
                nc.scalar.dma_start(out=qa_sb, in_=qa_d)
                nc.scalar.dma_start(out=cin_t[:, HC * C :], in_=c0N_d)
            else:
                nc.gpsimd.dma_start(out=cin_t, in_=cin_d[b])
            ctxT_t = cin_t[:, 0 : HC * C].rearrange("p (j c) -> p j c", j=HC)
            ctx_t = cin_t[:, HC * C :].rearrange("p (t h) -> p t h", t=CT)
            return ctxT_t, ctx_t

        tiles = {b: load_batch(b) for b in range(BPC)}

        # cross-iteration state of batch b-1: (b, e_t, e_sb, rq, ctx_t, oB_t)
        prev = None

        def emit_t_phase(state, tz):
            """T = S_bar_bar^T @ ctx into tz[:,0:H]; ts = T * rq."""
            bp, e_tp, e_sbp, rqp, ctx_tp, _ = state
            for t in range(CT):
                nc.tensor.matmul(
                    tz[:, 0:H],
                    lhsT=e_sbp[:, t // 4, t % 4, :],
                    rhs=ctx_tp[:, t, :],
                    start=(t == 0),
                    stop=(t == CT - 1),
                )
            ts = p_vec.tile([P, H], bf16, tag="ts", name=f"ts{bp}")
            nc.vector.tensor_scalar_mul(ts, tz[:, 0:H], rqp)
            return ts

        def emit_b_phase(state, ts, pairwise_store):
            """B pairs + ctx*B (fp8 out) on DVE, then store oB."""
            bp, e_tp, _, _, ctx_tp, oB_tp = state
            for p2 in range(CT // 2):
                t0 = 2 * p2
                pb = pp_ab.tile([P, 2, H], f32, tag="ab", name=f"pb{bp}{p2}")
                for k in range(2):
                    nc.tensor.matmul(
                        pb[:, k, :],
                        lhsT=e_tp[:, (t0 + k) * P : (t0 + k + 1) * P],
                        rhs=ts,
                        start=True,
                        stop=True,
                    )
                nc.vector.tensor_mul(
                    oB_tp[:, t0 : t0 + 2, :], ctx_tp[:, t0 : t0 + 2, :], pb
                )
                if pairwise_store:
                    nc.sync.dma_start(
                        out=oB_d[bp, :, t0 * H : (t0 + 2) * H],
                        in_=oB_tp[:, t0 : t0 + 2, :].rearrange("p t h -> p (t h)"),
                    )
            if not pairwise_store:
                nc.sync.dma_start(
                    out=oB_d[bp], in_=oB_tp.rearrange("p t h -> p (t h)")
                )

        for b in range(BPC):
            ctxT_t, ctx_t = tiles[b]
            bq = slice(b * Q, (b + 1) * Q)
            s1_b = wv[:, b : b + 1]
            last = b == BPC - 1

            # ---- S^T chunks into one 2-bank PSUM tile; ONE merged exp with
            # s1 bias whose accumulator IS zq; raw evac (+s1) split DVE/ACT ----
            e_t = p_et.tile([P, C], bf16, tag="e_t")
            st_raw = p_sr.tile([P, C], bf16, tag="st_raw")
            zq = p_vec.tile([P, 1], f32, tag="zq")
            stp = pp_st.tile([P, NCC, CCH], f32, tag="stp")
            for cc in range(NCC):
                for j in range(HC):
                    nc.tensor.matmul(
                        stp[:, cc, :],
                        lhsT=qt_cq[:, j, bq],
                        rhs=ctxT_t[:, j, cc * CCH : (cc + 1) * CCH],
                        start=(j == 0),
                        stop=(j == HC - 1),
                    )
            nc.scalar.activation(
                e_t, stp.rearrange("p n c -> p (n c)"), AF.Exp,
                bias=s1_b, accum_out=zq,
            )
            nc.vector.tensor_scalar_add(st_raw[:, 0:CCH], stp[:, 0], s1_b)
            nc.scalar.activation(
                st_raw[:, CCH:], stp[:, 1], AF.Identity, bias=s1_b, scale=1.0
            )
            rq = p_vec.tile([P, 1], f32, tag="rq")
            nc.vector.reciprocal(rq, zq)

            # shared tz bank: T(b-1) in [:,0:H], zc(b) in [:,H:H+CT]
            tz = pp_tz.tile([P, 264], f32, tag="tz", name=f"tz{b}")

            # ---- deferred T-phase of batch b-1 ----
            ts_prev = emit_t_phase(prev, tz) if prev is not None else None

            oAC_t = p_oAC.tile([P, 2, CT, H], bf16, tag="oAC")
            oA_t = oAC_t[:, 0]
            oCA_t = oAC_t[:, 1]

            def emit_a_phase():
                # A = S_raw @ qry per c-tile pair; evac on ACT, ctx*A on Pool
                for p2 in range(CT // 2):
                    t0 = 2 * p2
                    pa = pp_ab.tile([P, 2, H], f32, tag="ab", name=f"pa{b}{p2}")
                    for k in range(2):
                        nc.tensor.matmul(
                            pa[:, k, :],
                            lhsT=st_raw[:, (t0 + k) * P : (t0 + k + 1) * P],
                            rhs=qa_all[:, b, :],
                            start=True,
                            stop=True,
                        )
                    nc.scalar.copy(oA_t[:, t0 : t0 + 2, :], pa)
                    nc.gpsimd.tensor_mul(
                        oCA_t[:, t0 : t0 + 2, :],
                        ctx_t[:, t0 : t0 + 2, :],
                        oA_t[:, t0 : t0 + 2, :],
                    )

            def emit_tr_es():
                # transposes of e_t (two 4-tile waves) + zc ones-matmuls on
                # PE; e_sb = tr * (1/zc) via broadcast TT, one op per wave
                e_sb = p_esb.tile([P, NCC, 4, P], bf16, tag="e_sb")
                rc8 = p_vec.tile([P, NCC, 4], f32, tag="rc8")
                for w in range(NCC):
                    tr4 = pp_tr.tile([P, 4, P], bf16, tag="tr8", name=f"tr{b}{w}")
                    for t in range(4):
                        tt = 4 * w + t
                        nc.tensor.matmul(
                            tr4[:, t, :],
                            lhsT=e_t[:, tt * P : (tt + 1) * P],
                            rhs=ident,
                            is_transpose=True,
                        )
                        nc.tensor.matmul(
                            tz[:, H + tt : H + tt + 1],
                            lhsT=e_t[:, tt * P : (tt + 1) * P],
                            rhs=ones_col,
                            start=True,
                            stop=True,
                        )
                    nc.vector.reciprocal(rc8[:, w], tz[:, H + 4 * w : H + 4 * w + 4])
                    in0, in1 = broadcast_tensor_aps(tr4, rc8[:, w, :, None])
                    nc.vector.tensor_tensor(e_sb[:, w], in0, in1, ALU.mult)
                return e_sb

            if last:
                # last batch: es chain first, then T/ts of THIS batch
                # immediately (folds the epilogue into the loop), then A
                e_sb = emit_tr_es()
                state_last = (b, e_t, e_sb, rq, ctx_t, None)
                tzL = pp_tz.tile([P, 264], f32, tag="tz", name="tzL")
                ts_self = emit_t_phase(state_last, tzL)
                emit_a_phase()
            else:
                emit_a_phase()
                e_sb = emit_tr_es()

            # ---- deferred B-phase + ctxB store of batch b-1 ----
            if prev is not None:
                emit_b_phase(prev, ts_prev, pairwise_store=False)

            # ---- store A|ctxA of batch b (ready after the Pool products) ----
            nc.sync.dma_start(
                out=oAC_d[b], in_=oAC_t.rearrange("p u t h -> p (u t h)")
            )

            oB_t = p_oB.tile([P, CT, H], fp8, tag="oB")
            prev = (b, e_t, e_sb, rq, ctx_t, oB_t)

        # ---- tail: only B/ctxB/stores of the last batch remain ----
        emit_b_phase(prev, ts_self, pairwise_store=True)

    nc.compile()
    return nc


def _get_nc():
    if "nc" not in _NC_CACHE:
        _NC_CACHE["nc"] = _build_kernel()
    return _NC_CACHE["nc"]


def make_in_maps(context, query, c_weight, q_weight, cq_weight, bias):
    import ml_dtypes

    bf16 = ml_dtypes.bfloat16
    context = np.ascontiguousarray(np.asarray(context, dtype=np.float32))
    query = np.asarray(query, dtype=np.float32)
    cw = np.asarray(c_weight, dtype=np.float32).reshape(H)
    qw = np.asarray(q_weight, dtype=np.float32).reshape(H)
    cqw = np.asarray(cq_weight, dtype=np.float32).reshape(H)
    bs = float(np.asarray(bias, dtype=np.float32).reshape(1)[0])

    idones = np.concatenate(
        [np.eye(P, dtype=np.float32), np.ones((P, 1), np.float32)], axis=1
    ).astype(bf16)

    in_maps = []
    for i in range(N_CORES):
        sl = slice(i * BPC, (i + 1) * BPC)
        ctx_i = context[sl]
        qry_i = query[sl]
        # merged input: [ctxT | ctx] per batch
        # ctxT: [b, h, c] -> [b, p, j, c] with h = j*128+p
        ctxT_s = (
            ctx_i.transpose(0, 2, 1)
            .reshape(BPC, HC, P, C)
            .transpose(0, 2, 1, 3)
            .reshape(BPC, P, HC * C)
        )
        # ctx: [b, c, h] -> [b, p, t, h] with c = t*128+p
        ctx_s = (
            ctx_i.reshape(BPC, CT, P, H).transpose(0, 2, 1, 3).reshape(BPC, P, CT * H)
        )
        cin = np.ascontiguousarray(
            np.concatenate([ctxT_s, ctx_s], axis=2)
        ).astype(bf16)
        # qt_cq: host-folded (qry^T * cqw + cw), [p, j, b, q]
        qt_cq = (qry_i * cqw[None, None, :] + cw[None, None, :]).astype(np.float32)
        qt_s = (
            qt_cq.transpose(0, 2, 1)
            .reshape(BPC, HC, P, Q)
            .transpose(2, 1, 0, 3)
            .reshape(P, HC * BPC * Q)
        ).astype(bf16)
        # s1 columns: qry@qw + bias, [q, b] f32
        s1 = (qry_i @ qw + bs).astype(np.float32).T  # [q, b] -> wait: (b,q)->T=(q,b)
        # qry: [b, q, h] -> [q, b, h]
        qa_s = qry_i.transpose(1, 0, 2).reshape(P, BPC * H).astype(bf16)
        qtw = np.ascontiguousarray(np.concatenate([qt_s, idones], axis=1))
        in_maps.append(
            {
                "cin": cin,
                "c0T": np.ascontiguousarray(cin[0, :, 0 : HC * C]),
                "c0N": np.ascontiguousarray(cin[0, :, HC * C :]),
                "qtw": qtw,
                "qa": np.ascontiguousarray(qa_s),
                "wv": np.ascontiguousarray(s1),
            }
        )
    return in_maps


def kernel(context, query, c_mask, q_mask, c_weight, q_weight, cq_weight, bias):
    from concourse import bass_utils

    nc = _get_nc()
    in_maps = make_in_maps(context, query, c_weight, q_weight, cq_weight, bias)
    res = bass_utils.run_bass_kernel_spmd(nc, in_maps, core_ids=list(range(N_CORES)))

    context = np.asarray(context, dtype=np.float32)
    full = np.empty((B, C, 4 * H), dtype=np.float32)
    full[:, :, 0:H] = context

    for i in range(N_CORES):
        sl = slice(i * BPC, (i + 1) * BPC)
        oAC = (
            res.results[i]["oAC"]
            .reshape(BPC, P, 2, CT, H)
            .transpose(0, 2, 3, 1, 4)
            .reshape(BPC, 2, C, H)
            .astype(np.float32)
        )
        full[sl, :, H : 2 * H] = oAC[:, 0]
        full[sl, :, 2 * H : 3 * H] = oAC[:, 1]
        full[sl, :, 3 * H :] = (
            res.results[i]["oB"]
            .reshape(BPC, P, CT, H)
            .transpose(0, 2, 1, 3)
            .reshape(BPC, C, H)
            .astype(np.float32)
        )
    return full


# revision 11
# speedup vs baseline: 1.0244x; 1.0244x over previous
"""Trainium2 Bass kernel for ContextQueryAttention (BiDAF-style trilinear
attention). Data-parallel over batch across 8 NeuronCores (4 batches/core).

Per batch (c=1024 context rows, q=128 query rows, h=256 hidden):
  S[c,q]   = ctx@cw + (qry@qw)^T + (ctx*cqw)@qry^T + bias
  S_bar    = softmax_c(S); S_bar_bar = softmax_q(S)
  A        = S @ qry
  B        = S_bar @ (S_bar_bar^T @ ctx)
  out      = concat([ctx, A, ctx*A, ctx*B], -1)

v12: merged exp + late-batch reorder on v10.
  - qt_cq (= qry^T*cqw + cw) and s1 (= qry@qw + bias) precomputed on the
    host during input packing: no on-device preamble compute at all.
  - s1 applied as per-partition activation bias on the exp AND as a
    scalar add on the raw-S evac (kills the rank-1 rider matmuls).
  - zc (softmax_q denominators) via 8 tiny ones-column matmuls on the PE
    (kills the DVE reduce); e_sb = tr * rc via broadcast tensor_tensor
    (stride-0 in1), one op per 4-tile transpose wave.
  - ctx*B stored as fp8 (values are convex-combos of ctx, |x| << 240).
  - A, ctxA, ctxB in three separate contiguous HBM tensors; A stored
    right after its ACT evac, ctxA after Pool, ctxB after the deferred
    B phase (pairwise for the last batch to shorten the tail).
  - ALL inputs load up-front as back-to-back 1MB DMAs on the sync ring
    (batch 0 split in two contiguous tensors so compute starts sooner);
    every store queues behind them on the same ring, so the ring never
    mixes read/write directions at the HBM.
  - PE warmup: 7 dense N=512 matmuls on memset garbage (no DMA dep) so
    HAM un-throttles (1.2 -> 2.4 GHz) right as the first real matmul
    issues.
  - T/B of the last batch folded into the last iteration so only the
    final ctx*B products + stores trail the loop.
"""

import numpy as np

B, C, Q, H = 32, 1024, 128, 256
N_CORES = 8
BPC = B // N_CORES  # batches per core
P = 128
HC = H // P  # h chunks of 128
CT = C // P  # c tiles of 128
CCH = 512  # S^T free-dim chunk (1 PSUM bank of fp32)
NCC = C // CCH

_NC_CACHE = {}


def _build_kernel():
    import concourse.bacc as bacc
    import concourse.tile as tile
    from concourse import mybir
    from concourse.bass import broadcast_tensor_aps

    f32 = mybir.dt.float32
    bf16 = mybir.dt.bfloat16
    fp8 = mybir.dt.float8e4
    AF = mybir.ActivationFunctionType
    ALU = mybir.AluOpType

    nc = bacc.Bacc(trn_type="TRN2", target_bir_lowering=False, debug=False)
    # consts: qtw = [qt_cq cols (host-folded) | identity | ones-col] bf16
    QTW_W = HC * BPC * Q + P + 1
    qtw_d = nc.dram_tensor("qtw", [P, QTW_W], bf16, kind="ExternalInput").ap()
    qa_d = nc.dram_tensor("qa", [P, BPC * H], bf16, kind="ExternalInput").ap()
    # wv: per-batch s1 columns (qry@qw + bias), f32
    wv_d = nc.dram_tensor("wv", [P, BPC], f32, kind="ExternalInput").ap()
    # per-batch input: [ctxT cols | ctx cols] merged, 8KB/partition
    cin_d = nc.dram_tensor(
        "cin", [BPC, P, HC * C + CT * H], bf16, kind="ExternalInput"
    ).ap()
    # batch-0 input again as two separate contiguous tensors (fast startup)
    c0T_d = nc.dram_tensor("c0T", [P, HC * C], bf16, kind="ExternalInput").ap()
    c0N_d = nc.dram_tensor("c0N", [P, CT * H], bf16, kind="ExternalInput").ap()
    # outputs: A|ctxA merged per batch (one 1MB store), ctxB separate fp8
    oAC_d = nc.dram_tensor(
        "oAC", [BPC, P, 2 * CT * H], bf16, kind="ExternalOutput"
    ).ap()
    oB_d = nc.dram_tensor("oB", [BPC, P, CT * H], fp8, kind="ExternalOutput").ap()

    from contextlib import ExitStack

    with tile.TileContext(nc) as tc, ExitStack() as es:
        consts = es.enter_context(tc.tile_pool(name="consts", bufs=1))
        p_cin = es.enter_context(tc.tile_pool(name="p_cin", bufs=4))
        p_et = es.enter_context(tc.tile_pool(name="p_et", bufs=2))
        p_sr = es.enter_context(tc.tile_pool(name="p_sr", bufs=2))
        p_esb = es.enter_context(tc.tile_pool(name="p_esb", bufs=2))
        p_oAC = es.enter_context(tc.tile_pool(name="p_oAC", bufs=2))
        p_oB = es.enter_context(tc.tile_pool(name="p_oB", bufs=2))
        p_vec = es.enter_context(tc.tile_pool(name="p_vec", bufs=3))
        # PSUM banks: stp 2x[P,512]f32 (2) + tr 2x[P,4,128]bf16 (2)
        #             + tz 1x[P,264]f32 (1) + ab 3x[P,2,256]f32 (3)
        pp_st = es.enter_context(tc.tile_pool(name="pp_st", bufs=1, space="PSUM"))
        pp_tr = es.enter_context(tc.tile_pool(name="pp_tr", bufs=2, space="PSUM"))
        pp_tz = es.enter_context(tc.tile_pool(name="pp_tz", bufs=1, space="PSUM"))
        pp_ab = es.enter_context(tc.tile_pool(name="pp_ab", bufs=3, space="PSUM"))

        # ---- PE warmup spin on memset garbage (no DMA dependency): dense
        # transposes + N=512 matmuls (~4us) so HAM un-throttles early ----
        warm_src = consts.tile([P, P], bf16)
        nc.gpsimd.memset(warm_src, 1.0)
        warm_rhs = consts.tile([P, CCH], bf16)
        nc.vector.memset(warm_rhs, 1.0)
        for w in range(2):
            tr_warm = pp_tr.tile([P, 4, P], bf16, tag="tr8", name=f"warm{w}")
            for i in range(8):
                nc.tensor.matmul(
                    tr_warm[:, i % 4, :], lhsT=warm_src, rhs=warm_src,
                    is_transpose=True,
                )
        stp_w = pp_st.tile([P, NCC, CCH], f32, tag="stp", name="warm")
        for i in range(6):
            nc.tensor.matmul(
                stp_w[:, i % 2, :], lhsT=warm_src, rhs=warm_rhs, start=True, stop=True
            )

        # ---- const DMAs (qtw + s1 first; qa slots between batch-0 halves) ----
        qtw = consts.tile([P, QTW_W], bf16)
        nc.sync.dma_start(out=qtw, in_=qtw_d)
        wv = consts.tile([P, BPC], f32)
        nc.sync.dma_start(out=wv, in_=wv_d)
        qa_sb = consts.tile([P, BPC * H], bf16)
        qt_cq = qtw[:, 0 : HC * BPC * Q].rearrange("p (j bq) -> p j bq", j=HC)
        qa_all = qa_sb.rearrange("p (b h) -> p b h", b=BPC)
        idones = qtw[:, HC * BPC * Q :]  # [P, 129] = [I | 1]
        ident = idones[:, 0:P]
        ones_col = idones[:, P : P + 1]

        # ---- all input loads up-front, back-to-back on the sync ring:
        # the ring stays unidirectional (loads first, stores behind) ----
        def load_batch(b):
            cin_t = p_cin.tile([P, HC * C + CT * H], bf16, tag="cin", name=f"cin{b}")
            if b == 0:
                nc.sync.dma_start(out=cin_t[:, 0 : HC * C], in_=c0T_d)
                nc.sync.dma_start(out=qa_sb, in_=qa_d)
                nc.sync.dma_start(out=cin_t[:, HC * C :], in_=c0N_d)
            else:
                nc.sync.dma_start(out=cin_t, in_=cin_d[b])
            ctxT_t = cin_t[:, 0 : HC * C].rearrange("p (j c) -> p j c", j=HC)
            ctx_t = cin_t[:, HC * C :].rearrange("p (t h) -> p t h", t=CT)
            return ctxT_t, ctx_t

        tiles = {b: load_batch(b) for b in range(BPC)}

        # cross-iteration state of batch b-1: (b, e_t, e_sb, rq, ctx_t, oB_t)
        prev = None

        def emit_t_phase(state, tz):
            """T = S_bar_bar^T @ ctx into tz[:,0:H]; ts = T * rq."""
            bp, e_tp, e_sbp, rqp, ctx_tp, _ = state
            for t in range(CT):
                nc.tensor.matmul(
                    tz[:, 0:H],
                    lhsT=e_sbp[:, t // 4, t % 4, :],
                    rhs=ctx_tp[:, t, :],
                    start=(t == 0),
                    stop=(t == CT - 1),
                )
            ts = p_vec.tile([P, H], bf16, tag="ts", name=f"ts{bp}")
            nc.vector.tensor_scalar_mul(ts, tz[:, 0:H], rqp)
            return ts

        def emit_b_phase(state, ts, pairwise_store):
            """B pairs + ctx*B (fp8 out) on DVE, then store oB."""
            bp, e_tp, _, _, ctx_tp, oB_tp = state
            for p2 in range(CT // 2):
                t0 = 2 * p2
                pb = pp_ab.tile([P, 2, H], f32, tag="ab", name=f"pb{bp}{p2}")
                for k in range(2):
                    nc.tensor.matmul(
                        pb[:, k, :],
                        lhsT=e_tp[:, (t0 + k) * P : (t0 + k + 1) * P],
                        rhs=ts,
                        start=True,
                        stop=True,
                    )
                nc.vector.tensor_mul(
                    oB_tp[:, t0 : t0 + 2, :], ctx_tp[:, t0 : t0 + 2, :], pb
                )
                if pairwise_store:
                    nc.sync.dma_start(
                        out=oB_d[bp, :, t0 * H : (t0 + 2) * H],
                        in_=oB_tp[:, t0 : t0 + 2, :].rearrange("p t h -> p (t h)"),
                    )
            if not pairwise_store:
                nc.sync.dma_start(
                    out=oB_d[bp], in_=oB_tp.rearrange("p t h -> p (t h)")
                )

        for b in range(BPC):
            ctxT_t, ctx_t = tiles[b]
            bq = slice(b * Q, (b + 1) * Q)
            s1_b = wv[:, b : b + 1]
            last = b == BPC - 1

            # ---- S^T chunks into one 2-bank PSUM tile; ONE merged exp with
            # s1 bias whose accumulator IS zq; raw evac (+s1) split DVE/ACT ----
            e_t = p_et.tile([P, C], bf16, tag="e_t")
            st_raw = p_sr.tile([P, C], bf16, tag="st_raw")
            zq = p_vec.tile([P, 1], f32, tag="zq")
            stp = pp_st.tile([P, NCC, CCH], f32, tag="stp")
            for cc in range(NCC):
                for j in range(HC):
                    nc.tensor.matmul(
                        stp[:, cc, :],
                        lhsT=qt_cq[:, j, bq],
                        rhs=ctxT_t[:, j, cc * CCH : (cc + 1) * CCH],
                        start=(j == 0),
                        stop=(j == HC - 1),
                    )
            nc.scalar.activation(
                e_t, stp.rearrange("p n c -> p (n c)"), AF.Exp,
                bias=s1_b, accum_out=zq,
            )
            nc.vector.tensor_scalar_add(st_raw[:, 0:CCH], stp[:, 0], s1_b)
            nc.scalar.activation(
                st_raw[:, CCH:], stp[:, 1], AF.Identity, bias=s1_b, scale=1.0
            )
            rq = p_vec.tile([P, 1], f32, tag="rq")
            nc.vector.reciprocal(rq, zq)

            # shared tz bank: T(b-1) in [:,0:H], zc(b) in [:,H:H+CT]
            tz = pp_tz.tile([P, 264], f32, tag="tz", name=f"tz{b}")

            # ---- deferred T-phase of batch b-1 ----
            ts_prev = emit_t_phase(prev, tz) if prev is not None else None

            oAC_t = p_oAC.tile([P, 2, CT, H], bf16, tag="oAC")
            oA_t = oAC_t[:, 0]
            oCA_t = oAC_t[:, 1]

            def emit_a_phase():
                # A = S_raw @ qry per c-tile pair; evac on ACT, ctx*A on Pool
                for p2 in range(CT // 2):
                    t0 = 2 * p2
                    pa = pp_ab.tile([P, 2, H], f32, tag="ab", name=f"pa{b}{p2}")
                    for k in range(2):
                        nc.tensor.matmul(
                            pa[:, k, :],
                            lhsT=st_raw[:, (t0 + k) * P : (t0 + k + 1) * P],
                            rhs=qa_all[:, b, :],
                            start=True,
                            stop=True,
                        )
                    nc.scalar.copy(oA_t[:, t0 : t0 + 2, :], pa)
                    nc.gpsimd.tensor_mul(
                        oCA_t[:, t0 : t0 + 2, :],
                        ctx_t[:, t0 : t0 + 2, :],
                        oA_t[:, t0 : t0 + 2, :],
                    )

            def emit_tr_es():
                # transposes of e_t (two 4-tile waves) + zc ones-matmuls on
                # PE; e_sb = tr * (1/zc) via broadcast TT, one op per wave
                e_sb = p_esb.tile([P, NCC, 4, P], bf16, tag="e_sb")
                rc8 = p_vec.tile([P, NCC, 4], f32, tag="rc8")
                for w in range(NCC):
                    tr4 = pp_tr.tile([P, 4, P], bf16, tag="tr8", name=f"tr{b}{w}")
                    for t in range(4):
                        tt = 4 * w + t
                        nc.tensor.matmul(
                            tr4[:, t, :],
                            lhsT=e_t[:, tt * P : (tt + 1) * P],
                            rhs=ident,
                            is_transpose=True,
                        )
                        nc.tensor.matmul(
                            tz[:, H + tt : H + tt + 1],
                            lhsT=e_t[:, tt * P : (tt + 1) * P],
                            rhs=ones_col,
                            start=True,
                            stop=True,
                        )
                    nc.vector.reciprocal(rc8[:, w], tz[:, H + 4 * w : H + 4 * w + 4])
                    in0, in1 = broadcast_tensor_aps(tr4, rc8[:, w, :, None])
                    nc.vector.tensor_tensor(e_sb[:, w], in0, in1, ALU.mult)
                return e_sb

            emit_a_phase()
            e_sb = emit_tr_es()

            # ---- deferred B-phase + ctxB store of batch b-1 (for the last
            # iteration this also hides the e_sb latency before T(last)) ----
            if prev is not None:
                emit_b_phase(prev, ts_prev, pairwise_store=False)

            if last:
                # T/ts of the last batch inside the loop: only B/ctxB trail
                state_last = (b, e_t, e_sb, rq, ctx_t, None)
                tzL = pp_tz.tile([P, 264], f32, tag="tz", name="tzL")
                ts_self = emit_t_phase(state_last, tzL)

            # ---- store A|ctxA of batch b (ready after the Pool products) ----
            nc.sync.dma_start(
                out=oAC_d[b], in_=oAC_t.rearrange("p u t h -> p (u t h)")
            )

            oB_t = p_oB.tile([P, CT, H], fp8, tag="oB")
            prev = (b, e_t, e_sb, rq, ctx_t, oB_t)

        # ---- tail: only B/ctxB/stores of the last batch remain ----
        emit_b_phase(prev, ts_self, pairwise_store=True)

    nc.compile()
    return nc


def _get_nc():
    if "nc" not in _NC_CACHE:
        _NC_CACHE["nc"] = _build_kernel()
    return _NC_CACHE["nc"]


def make_in_maps(context, query, c_weight, q_weight, cq_weight, bias):
    import ml_dtypes

    bf16 = ml_dtypes.bfloat16
    context = np.ascontiguousarray(np.asarray(context, dtype=np.float32))
    query = np.asarray(query, dtype=np.float32)
    cw = np.asarray(c_weight, dtype=np.float32).reshape(H)
    qw = np.asarray(q_weight, dtype=np.float32).reshape(H)
    cqw = np.asarray(cq_weight, dtype=np.float32).reshape(H)
    bs = float(np.asarray(bias, dtype=np.float32).reshape(1)[0])

    idones = np.concatenate(
        [np.eye(P, dtype=np.float32), np.ones((P, 1), np.float32)], axis=1
    ).astype(bf16)

    in_maps = []
    for i in range(N_CORES):
        sl = slice(i * BPC, (i + 1) * BPC)
        ctx_i = context[sl]
        qry_i = query[sl]
        # merged input: [ctxT | ctx] per batch
        # ctxT: [b, h, c] -> [b, p, j, c] with h = j*128+p
        ctxT_s = (
            ctx_i.transpose(0, 2, 1)
            .reshape(BPC, HC, P, C)
            .transpose(0, 2, 1, 3)
            .reshape(BPC, P, HC * C)
        )
        # ctx: [b, c, h] -> [b, p, t, h] with c = t*128+p
        ctx_s = (
            ctx_i.reshape(BPC, CT, P, H).transpose(0, 2, 1, 3).reshape(BPC, P, CT * H)
        )
        cin = np.ascontiguousarray(
            np.concatenate([ctxT_s, ctx_s], axis=2)
        ).astype(bf16)
        # qt_cq: host-folded (qry^T * cqw + cw), [p, j, b, q]
        qt_cq = (qry_i * cqw[None, None, :] + cw[None, None, :]).astype(np.float32)
        qt_s = (
            qt_cq.transpose(0, 2, 1)
            .reshape(BPC, HC, P, Q)
            .transpose(2, 1, 0, 3)
            .reshape(P, HC * BPC * Q)
        ).astype(bf16)
        # s1 columns: qry@qw + bias, [q, b] f32
        s1 = (qry_i @ qw + bs).astype(np.float32).T  # [q, b] -> wait: (b,q)->T=(q,b)
        # qry: [b, q, h] -> [q, b, h]
        qa_s = qry_i.transpose(1, 0, 2).reshape(P, BPC * H).astype(bf16)
        qtw = np.ascontiguousarray(np.concatenate([qt_s, idones], axis=1))
        in_maps.append(
            {
                "cin": cin,
                "c0T": np.ascontiguousarray(cin[0, :, 0 : HC * C]),
                "c0N": np.ascontiguousarray(cin[0, :, HC * C :]),
                "qtw": qtw,
                "qa": np.ascontiguousarray(qa_s),
                "wv": np.ascontiguousarray(s1),
            }
        )
    return in_maps


def kernel(context, query, c_mask, q_mask, c_weight, q_weight, cq_weight, bias):
    from concourse import bass_utils

    nc = _get_nc()
    in_maps = make_in_maps(context, query, c_weight, q_weight, cq_weight, bias)
    res = bass_utils.run_bass_kernel_spmd(nc, in_maps, core_ids=list(range(N_CORES)))

    context = np.asarray(context, dtype=np.float32)
    full = np.empty((B, C, 4 * H), dtype=np.float32)
    full[:, :, 0:H] = context

    for i in range(N_CORES):
        sl = slice(i * BPC, (i + 1) * BPC)
        oAC = (
            res.results[i]["oAC"]
            .reshape(BPC, P, 2, CT, H)
            .transpose(0, 2, 3, 1, 4)
            .reshape(BPC, 2, C, H)
            .astype(np.float32)
        )
        full[sl, :, H : 2 * H] = oAC[:, 0]
        full[sl, :, 2 * H : 3 * H] = oAC[:, 1]
        full[sl, :, 3 * H :] = (
            res.results[i]["oB"]
            .reshape(BPC, P, CT, H)
            .transpose(0, 2, 1, 3)
            .reshape(BPC, C, H)
            .astype(np.float32)
        )
    return full


# revision 13
# speedup vs baseline: 1.1103x; 1.0839x over previous
"""Trainium2 Bass kernel for ContextQueryAttention (BiDAF-style trilinear
attention). Data-parallel over batch across 8 NeuronCores (4 batches/core).

Per batch (c=1024 context rows, q=128 query rows, h=256 hidden):
  S[c,q]   = ctx@cw + (qry@qw)^T + (ctx*cqw)@qry^T + bias
  S_bar    = softmax_c(S); S_bar_bar = softmax_q(S)
  A        = S @ qry
  B        = S_bar @ (S_bar_bar^T @ ctx)
  out      = concat([ctx, A, ctx*A, ctx*B], -1)

v13: v10 + split A/ctxA stores + A-first last iteration.
  - qt_cq (= qry^T*cqw + cw) and s1 (= qry@qw + bias) precomputed on the
    host during input packing: no on-device preamble compute at all.
  - s1 applied as per-partition activation bias on the exp AND as a
    scalar add on the raw-S evac (kills the rank-1 rider matmuls).
  - zc (softmax_q denominators) via 8 tiny ones-column matmuls on the PE
    (kills the DVE reduce); e_sb = tr * rc via broadcast tensor_tensor
    (stride-0 in1), one op per 4-tile transpose wave.
  - ctx*B stored as fp8 (values are convex-combos of ctx, |x| << 240).
  - A, ctxA, ctxB in three separate contiguous HBM tensors; A stored
    right after its ACT evac, ctxA after Pool, ctxB after the deferred
    B phase (pairwise for the last batch to shorten the tail).
  - ALL inputs load up-front as back-to-back 1MB DMAs on the sync ring
    (batch 0 split in two contiguous tensors so compute starts sooner);
    every store queues behind them on the same ring, so the ring never
    mixes read/write directions at the HBM.
  - PE warmup: 7 dense N=512 matmuls on memset garbage (no DMA dep) so
    HAM un-throttles (1.2 -> 2.4 GHz) right as the first real matmul
    issues.
  - T/B of the last batch folded into the last iteration so only the
    final ctx*B products + stores trail the loop.
"""

import numpy as np

B, C, Q, H = 32, 1024, 128, 256
N_CORES = 8
BPC = B // N_CORES  # batches per core
P = 128
HC = H // P  # h chunks of 128
CT = C // P  # c tiles of 128
CCH = 512  # S^T free-dim chunk (1 PSUM bank of fp32)
NCC = C // CCH

_NC_CACHE = {}


def _build_kernel():
    import concourse.bacc as bacc
    import concourse.tile as tile
    from concourse import mybir
    from concourse.bass import broadcast_tensor_aps

    f32 = mybir.dt.float32
    bf16 = mybir.dt.bfloat16
    fp8 = mybir.dt.float8e4
    AF = mybir.ActivationFunctionType
    ALU = mybir.AluOpType

    nc = bacc.Bacc(trn_type="TRN2", target_bir_lowering=False, debug=False)
    # consts: qtw = [qt_cq cols (host-folded) | identity | ones-col] bf16
    QTW_W = HC * BPC * Q + P + 1
    qtw_d = nc.dram_tensor("qtw", [P, QTW_W], bf16, kind="ExternalInput").ap()
    qa_d = nc.dram_tensor("qa", [P, BPC * H], bf16, kind="ExternalInput").ap()
    # wv: per-batch s1 columns (qry@qw + bias), f32
    wv_d = nc.dram_tensor("wv", [P, BPC], f32, kind="ExternalInput").ap()
    # per-batch input: [ctxT cols | ctx cols] merged, 8KB/partition
    cin_d = nc.dram_tensor(
        "cin", [BPC, P, HC * C + CT * H], bf16, kind="ExternalInput"
    ).ap()
    # batch-0 input again as two separate contiguous tensors (fast startup)
    c0T_d = nc.dram_tensor("c0T", [P, HC * C], bf16, kind="ExternalInput").ap()
    c0N_d = nc.dram_tensor("c0N", [P, CT * H], bf16, kind="ExternalInput").ap()
    # outputs: A|ctxA merged per batch (one 1MB store), ctxB separate fp8
    oAC_d = nc.dram_tensor(
        "oAC", [BPC, P, 2 * CT * H], bf16, kind="ExternalOutput"
    ).ap()
    oB_d = nc.dram_tensor("oB", [BPC, P, CT * H], fp8, kind="ExternalOutput").ap()

    from contextlib import ExitStack

    with tile.TileContext(nc) as tc, ExitStack() as es:
        consts = es.enter_context(tc.tile_pool(name="consts", bufs=1))
        p_cin = es.enter_context(tc.tile_pool(name="p_cin", bufs=4))
        p_et = es.enter_context(tc.tile_pool(name="p_et", bufs=2))
        p_sr = es.enter_context(tc.tile_pool(name="p_sr", bufs=2))
        p_esb = es.enter_context(tc.tile_pool(name="p_esb", bufs=2))
        p_oAC = es.enter_context(tc.tile_pool(name="p_oAC", bufs=2))
        p_oB = es.enter_context(tc.tile_pool(name="p_oB", bufs=2))
        p_vec = es.enter_context(tc.tile_pool(name="p_vec", bufs=3))
        # PSUM banks: stp 2x[P,512]f32 (2) + tr 2x[P,4,128]bf16 (2)
        #             + tz 1x[P,264]f32 (1) + ab 3x[P,2,256]f32 (3)
        pp_st = es.enter_context(tc.tile_pool(name="pp_st", bufs=2, space="PSUM"))
        pp_tr = es.enter_context(tc.tile_pool(name="pp_tr", bufs=2, space="PSUM"))
        pp_tz = es.enter_context(tc.tile_pool(name="pp_tz", bufs=1, space="PSUM"))
        pp_ab = es.enter_context(tc.tile_pool(name="pp_ab", bufs=3, space="PSUM"))

        # ---- PE warmup spin on memset garbage (no DMA dependency): dense
        # transposes + N=512 matmuls (~4us) so HAM un-throttles early ----
        warm_src = consts.tile([P, P], bf16)
        nc.gpsimd.memset(warm_src, 1.0)
        warm_rhs = consts.tile([P, CCH], bf16)
        nc.vector.memset(warm_rhs, 1.0)
        for w in range(2):
            tr_warm = pp_tr.tile([P, 4, P], bf16, tag="tr8", name=f"warm{w}")
            for i in range(8):
                nc.tensor.matmul(
                    tr_warm[:, i % 4, :], lhsT=warm_src, rhs=warm_src,
                    is_transpose=True,
                )
        for i in range(6):
            stp_w = pp_st.tile([P, CCH], f32, tag="stp", name=f"warm{i}")
            nc.tensor.matmul(stp_w, lhsT=warm_src, rhs=warm_rhs, start=True, stop=True)

        # ---- const DMAs (qtw + s1 first; qa slots between batch-0 halves) ----
        qtw = consts.tile([P, QTW_W], bf16)
        nc.sync.dma_start(out=qtw, in_=qtw_d)
        wv = consts.tile([P, BPC], f32)
        nc.sync.dma_start(out=wv, in_=wv_d)
        qa_sb = consts.tile([P, BPC * H], bf16)
        qt_cq = qtw[:, 0 : HC * BPC * Q].rearrange("p (j bq) -> p j bq", j=HC)
        qa_all = qa_sb.rearrange("p (b h) -> p b h", b=BPC)
        idones = qtw[:, HC * BPC * Q :]  # [P, 129] = [I | 1]
        ident = idones[:, 0:P]
        ones_col = idones[:, P : P + 1]

        # ---- all input loads up-front, back-to-back on the sync ring:
        # the ring stays unidirectional (loads first, stores behind) ----
        def load_batch(b):
            cin_t = p_cin.tile([P, HC * C + CT * H], bf16, tag="cin", name=f"cin{b}")
            if b == 0:
                nc.sync.dma_start(out=cin_t[:, 0 : HC * C], in_=c0T_d)
                nc.sync.dma_start(out=qa_sb, in_=qa_d)
                nc.sync.dma_start(out=cin_t[:, HC * C :], in_=c0N_d)
            else:
                nc.sync.dma_start(out=cin_t, in_=cin_d[b])
            ctxT_t = cin_t[:, 0 : HC * C].rearrange("p (j c) -> p j c", j=HC)
            ctx_t = cin_t[:, HC * C :].rearrange("p (t h) -> p t h", t=CT)
            return ctxT_t, ctx_t

        tiles = {b: load_batch(b) for b in range(BPC)}

        # cross-iteration state of batch b-1: (b, e_t, e_sb, rq, ctx_t, oB_t)
        prev = None

        def emit_t_phase(state, tz):
            """T = S_bar_bar^T @ ctx into tz[:,0:H]; ts = T * rq."""
            bp, e_tp, e_sbp, rqp, ctx_tp, _ = state
            for t in range(CT):
                nc.tensor.matmul(
                    tz[:, 0:H],
                    lhsT=e_sbp[:, t // 4, t % 4, :],
                    rhs=ctx_tp[:, t, :],
                    start=(t == 0),
                    stop=(t == CT - 1),
                )
            ts = p_vec.tile([P, H], bf16, tag="ts", name=f"ts{bp}")
            nc.vector.tensor_scalar_mul(ts, tz[:, 0:H], rqp)
            return ts

        def emit_b_phase(state, ts, pairwise_store):
            """B pairs + ctx*B (fp8 out) on DVE, then store oB."""
            bp, e_tp, _, _, ctx_tp, oB_tp = state
            for p2 in range(CT // 2):
                t0 = 2 * p2
                pb = pp_ab.tile([P, 2, H], f32, tag="ab", name=f"pb{bp}{p2}")
                for k in range(2):
                    nc.tensor.matmul(
                        pb[:, k, :],
                        lhsT=e_tp[:, (t0 + k) * P : (t0 + k + 1) * P],
                        rhs=ts,
                        start=True,
                        stop=True,
                    )
                nc.vector.tensor_mul(
                    oB_tp[:, t0 : t0 + 2, :], ctx_tp[:, t0 : t0 + 2, :], pb
                )
                if pairwise_store:
                    nc.sync.dma_start(
                        out=oB_d[bp, :, t0 * H : (t0 + 2) * H],
                        in_=oB_tp[:, t0 : t0 + 2, :].rearrange("p t h -> p (t h)"),
                    )
            if not pairwise_store:
                nc.sync.dma_start(
                    out=oB_d[bp], in_=oB_tp.rearrange("p t h -> p (t h)")
                )

        for b in range(BPC):
            ctxT_t, ctx_t = tiles[b]
            bq = slice(b * Q, (b + 1) * Q)
            s1_b = wv[:, b : b + 1]
            last = b == BPC - 1

            # ---- S^T chunks; exp with s1 bias + fused row-sums; raw evac
            # (+s1) split ACT/DVE ----
            e_t = p_et.tile([P, C], bf16, tag="e_t")
            st_raw = p_sr.tile([P, C], bf16, tag="st_raw")
            rsum = p_vec.tile([P, NCC], f32, tag="rsum")
            for cc in range(NCC):
                sl = slice(cc * CCH, (cc + 1) * CCH)
                stp = pp_st.tile([P, CCH], f32, tag="stp")
                for j in range(HC):
                    nc.tensor.matmul(
                        stp,
                        lhsT=qt_cq[:, j, bq],
                        rhs=ctxT_t[:, j, sl],
                        start=(j == 0),
                        stop=(j == HC - 1),
                    )
                nc.scalar.activation(
                    e_t[:, sl],
                    stp,
                    AF.Exp,
                    bias=s1_b,
                    accum_out=rsum[:, cc : cc + 1],
                )
                if cc == 0:
                    nc.vector.tensor_scalar_add(st_raw[:, sl], stp, s1_b)
                else:
                    nc.scalar.activation(
                        st_raw[:, sl], stp, AF.Identity, bias=s1_b, scale=1.0
                    )

            # softmax_c denominators: zq = rsum0+rsum1 (Pool), rq = 1/zq (DVE)
            zq = p_vec.tile([P, 1], f32, tag="zq")
            nc.gpsimd.tensor_add(zq, rsum[:, 0:1], rsum[:, 1:2])
            rq = p_vec.tile([P, 1], f32, tag="rq")
            nc.vector.reciprocal(rq, zq)

            # shared tz bank: T(b-1) in [:,0:H], zc(b) in [:,H:H+CT]
            tz = pp_tz.tile([P, 264], f32, tag="tz", name=f"tz{b}")

            # ---- deferred T-phase of batch b-1 ----
            ts_prev = emit_t_phase(prev, tz) if prev is not None else None

            oAC_t = p_oAC.tile([P, 2, CT, H], bf16, tag="oAC")
            oA_t = oAC_t[:, 0]
            oCA_t = oAC_t[:, 1]

            def emit_a_phase():
                # A = S_raw @ qry per c-tile pair; evac on ACT, ctx*A on Pool
                for p2 in range(CT // 2):
                    t0 = 2 * p2
                    pa = pp_ab.tile([P, 2, H], f32, tag="ab", name=f"pa{b}{p2}")
                    for k in range(2):
                        nc.tensor.matmul(
                            pa[:, k, :],
                            lhsT=st_raw[:, (t0 + k) * P : (t0 + k + 1) * P],
                            rhs=qa_all[:, b, :],
                            start=True,
                            stop=True,
                        )
                    nc.scalar.copy(oA_t[:, t0 : t0 + 2, :], pa)
                    nc.gpsimd.tensor_mul(
                        oCA_t[:, t0 : t0 + 2, :],
                        ctx_t[:, t0 : t0 + 2, :],
                        oA_t[:, t0 : t0 + 2, :],
                    )
                # A half ships as soon as the ACT evacs are done
                nc.sync.dma_start(
                    out=oAC_d[b, :, 0 : CT * H],
                    in_=oA_t.rearrange("p t h -> p (t h)"),
                )

            def emit_tr_es():
                # transposes of e_t (two 4-tile waves) + zc ones-matmuls on
                # PE; e_sb = tr * (1/zc) via broadcast TT, one op per wave
                e_sb = p_esb.tile([P, NCC, 4, P], bf16, tag="e_sb")
                rc8 = p_vec.tile([P, NCC, 4], f32, tag="rc8")
                for w in range(NCC):
                    tr4 = pp_tr.tile([P, 4, P], bf16, tag="tr8", name=f"tr{b}{w}")
                    for t in range(4):
                        tt = 4 * w + t
                        nc.tensor.matmul(
                            tr4[:, t, :],
                            lhsT=e_t[:, tt * P : (tt + 1) * P],
                            rhs=ident,
                            is_transpose=True,
                        )
                        nc.tensor.matmul(
                            tz[:, H + tt : H + tt + 1],
                            lhsT=e_t[:, tt * P : (tt + 1) * P],
                            rhs=ones_col,
                            start=True,
                            stop=True,
                        )
                    nc.vector.reciprocal(rc8[:, w], tz[:, H + 4 * w : H + 4 * w + 4])
                    in0, in1 = broadcast_tensor_aps(tr4, rc8[:, w, :, None])
                    nc.vector.tensor_tensor(e_sb[:, w], in0, in1, ALU.mult)
                return e_sb

            emit_a_phase()
            e_sb = emit_tr_es()

            # ---- deferred B-phase + ctxB store of batch b-1 (for the last
            # iteration this also hides the e_sb latency before T(last)) ----
            if prev is not None:
                emit_b_phase(prev, ts_prev, pairwise_store=False)

            if last:
                # T/ts of the last batch inside the loop: only B/ctxB trail
                state_last = (b, e_t, e_sb, rq, ctx_t, None)
                tzL = pp_tz.tile([P, 264], f32, tag="tz", name="tzL")
                ts_self = emit_t_phase(state_last, tzL)

            # ---- store the ctxA half of batch b (after the Pool products) ----
            nc.sync.dma_start(
                out=oAC_d[b, :, CT * H :],
                in_=oCA_t.rearrange("p t h -> p (t h)"),
            )

            oB_t = p_oB.tile([P, CT, H], fp8, tag="oB")
            prev = (b, e_t, e_sb, rq, ctx_t, oB_t)

        # ---- tail: only B/ctxB/stores of the last batch remain ----
        emit_b_phase(prev, ts_self, pairwise_store=True)

    nc.compile()
    return nc


def _get_nc():
    if "nc" not in _NC_CACHE:
        _NC_CACHE["nc"] = _build_kernel()
    return _NC_CACHE["nc"]


def make_in_maps(context, query, c_weight, q_weight, cq_weight, bias):
    import ml_dtypes

    bf16 = ml_dtypes.bfloat16
    context = np.ascontiguousarray(np.asarray(context, dtype=np.float32))
    query = np.asarray(query, dtype=np.float32)
    cw = np.asarray(c_weight, dtype=np.float32).reshape(H)
    qw = np.asarray(q_weight, dtype=np.float32).reshape(H)
    cqw = np.asarray(cq_weight, dtype=np.float32).reshape(H)
    bs = float(np.asarray(bias, dtype=np.float32).reshape(1)[0])

    idones = np.concatenate(
        [np.eye(P, dtype=np.float32), np.ones((P, 1), np.float32)], axis=1
    ).astype(bf16)

    in_maps = []
    for i in range(N_CORES):
        sl = slice(i * BPC, (i + 1) * BPC)
        ctx_i = context[sl]
        qry_i = query[sl]
        # merged input: [ctxT | ctx] per batch
        # ctxT: [b, h, c] -> [b, p, j, c] with h = j*128+p
        ctxT_s = (
            ctx_i.transpose(0, 2, 1)
            .reshape(BPC, HC, P, C)
            .transpose(0, 2, 1, 3)
            .reshape(BPC, P, HC * C)
        )
        # ctx: [b, c, h] -> [b, p, t, h] with c = t*128+p
        ctx_s = (
            ctx_i.reshape(BPC, CT, P, H).transpose(0, 2, 1, 3).reshape(BPC, P, CT * H)
        )
        cin = np.ascontiguousarray(
            np.concatenate([ctxT_s, ctx_s], axis=2)
        ).astype(bf16)
        # qt_cq: host-folded (qry^T * cqw + cw), [p, j, b, q]
        qt_cq = (qry_i * cqw[None, None, :] + cw[None, None, :]).astype(np.float32)
        qt_s = (
            qt_cq.transpose(0, 2, 1)
            .reshape(BPC, HC, P, Q)
            .transpose(2, 1, 0, 3)
            .reshape(P, HC * BPC * Q)
        ).astype(bf16)
        # s1 columns: qry@qw + bias, [q, b] f32
        s1 = (qry_i @ qw + bs).astype(np.float32).T  # [q, b] -> wait: (b,q)->T=(q,b)
        # qry: [b, q, h] -> [q, b, h]
        qa_s = qry_i.transpose(1, 0, 2).reshape(P, BPC * H).astype(bf16)
        qtw = np.ascontiguousarray(np.concatenate([qt_s, idones], axis=1))
        in_maps.append(
            {
                "cin": cin,
                "c0T": np.ascontiguousarray(cin[0, :, 0 : HC * C]),
                "c0N": np.ascontiguousarray(cin[0, :, HC * C :]),
                "qtw": qtw,
                "qa": np.ascontiguousarray(qa_s),
                "wv": np.ascontiguousarray(s1),
            }
        )
    return in_maps


def kernel(context, query, c_mask, q_mask, c_weight, q_weight, cq_weight, bias):
    from concourse import bass_utils

    nc = _get_nc()
    in_maps = make_in_maps(context, query, c_weight, q_weight, cq_weight, bias)
    res = bass_utils.run_bass_kernel_spmd(nc, in_maps, core_ids=list(range(N_CORES)))

    context = np.asarray(context, dtype=np.float32)
    full = np.empty((B, C, 4 * H), dtype=np.float32)
    full[:, :, 0:H] = context

    for i in range(N_CORES):
        sl = slice(i * BPC, (i + 1) * BPC)
        oAC = (
            res.results[i]["oAC"]
            .reshape(BPC, P, 2, CT, H)
            .transpose(0, 2, 3, 1, 4)
            .reshape(BPC, 2, C, H)
            .astype(np.float32)
        )
        full[sl, :, H : 2 * H] = oAC[:, 0]
        full[sl, :, 2 * H : 3 * H] = oAC[:, 1]
        full[sl, :, 3 * H :] = (
            res.results[i]["oB"]
            .reshape(BPC, P, CT, H)
            .transpose(0, 2, 1, 3)
            .reshape(BPC, C, H)
            .astype(np.float32)
        )
    return full
